# revision 1
# baseline (speedup 1.0000x reference)
"""Trainium2 Bass kernel for nn_Model_17085379903564 (HiPPO-LegT multiscale
spectral forecaster).

Math: the reference normalizes x per (b,e) series, runs a HiPPO-LegT scan,
takes 32 rFFT modes of the state trajectory, mixes modes with complex
weights w, evaluates the irFFT at t=511, projects on Legendre polynomials
(Em), mixes two scales with an MLP, and un-normalizes.

Everything from the input to the Legendre projection is LINEAR with
constant coefficients, so per scale (L = 512 or 1024) the whole chain
collapses to one dense operator W2 folded from the scan kernel, the DFT
and the point-irFFT weights.  W2 is factored by SVD, W2 ~= U @ V; with
g = f.T @ U and P = V @ w (x-independent), xdc = g @ P.  Empirically the
real input/weight distributions excite only ~the top 128 singular
directions, so RANK=128 loses nothing (validated to ~1e-4 in fp64).

Quantization: V rows are scaled to fp8 range with per-row factors folded
into U (exact algebra); w is scaled globally per scale with the factor
folded into the host-passed mlp weight.  P is then computed with fp8
DoubleRow matmuls (2x PE throughput, half the DMA bytes).  The instance
norm commutes through: with raw x, xdc_raw = g@P, and the correction
  xdcT = P.T g.T - tp mu.T,  tp = (colsum U) @ P
is a rank-1 matmul; std scaling cancels until the final affine
  out = dec * std + (b*std + mean),
which also restores the mean.  mean/var are computed on-device from an
f32 copy of x (bf16-derived stats are NOT accurate enough: the mean
error lands directly on the output).

Sharding (8 cores): V/w sharded over the spectral dim n (32 of 256 rows
per core) -> per-core partial P -> partial dec; host sums the 8 partial
decs (collectives cost more than they save at this size).  Everything
else is replicated.

All DRAM operands are pre-swizzled on the host into partition-major
[128, N] layouts so every DMA line is 1.5-4KB contiguous; the fp8
V|w blocks are interleaved per k-subtile and split into 4 chunks per
scale so P matmuls start as soon as the first chunk lands.
"""

from contextlib import ExitStack

import ml_dtypes
import numpy as np

import concourse.bacc as bacc
import concourse.bass as bass
import concourse.mybir as mybir
import concourse.tile as tile
from concourse.bass_utils import run_bass_kernel_spmd

# ---- problem constants (hardcoded; kernel.py must be self-contained) ----
B_SZ = 4
SEQ_LEN = 1024
PRED_LEN = 512
E_IN = 32
N_ORD = 256
MODES = 32
MULTISCALE = (1, 2)
BE = B_SZ * E_IN            # 128
N_CORES = 8
NSL = N_ORD // N_CORES      # 32  n-rows per core
NK = 2 * NSL * MODES        # 2048 contraction length per core (re+im)
KSUB = NK // 128            # 16 k-subtiles
NCHUNK = 4                  # weight DMA chunks per scale
RANK = 128                  # SVD rank kept for the W2 operators

F32 = mybir.dt.float32
BF16 = mybir.dt.bfloat16
FP8 = mybir.dt.float8e4
BF16_NP = np.dtype(ml_dtypes.bfloat16)
FP8_NP = np.dtype(ml_dtypes.float8_e4m3)


# ---------------------------------------------------------------- constants
def _transition_lmu(N):
    Q = np.arange(N, dtype=np.float64)
    R = (2 * Q + 1)[:, None]
    j, i = np.meshgrid(Q, Q)
    A = np.where(i < j, -1.0, (-1.0) ** (i - j + 1)) * R
    Bv = ((-1.0) ** Q[:, None] * R)[:, 0]
    return A, Bv


def _bilinear(A, Bv, dt):
    I = np.eye(A.shape[0])
    M = I - (dt / 2.0) * A
    Ad = np.linalg.solve(M, I + (dt / 2.0) * A)
    Bd = np.linalg.solve(M, dt * Bv)
    return Ad, Bd


def _legendre_vander(x, N):
    P = np.zeros((N, x.shape[0]))
    P[0] = 1.0
    if N > 1:
        P[1] = x
    for n in range(1, N - 1):
        P[n + 1] = ((2 * n + 1) * x * P[n] - n * P[n - 1]) / (n + 1)
    return P.T


def _scale_consts(ms):
    """Per-scale constants: swizzled u/emt/su plus the fp8 V row-blocks."""
    L = ms * PRED_LEN
    A, Bv = _transition_lmu(N_ORD)
    Ad, Bd = _bilinear(A, Bv, 1.0 / L)
    vals = np.arange(0.0, 1.0, 1.0 / L)
    Em = _legendre_vander(1.0 - 2.0 * vals, N_ORD)        # (L, N)

    G = np.empty((L, N_ORD))
    g = Bd.copy()
    for m in range(L):
        G[m] = g
        g = Ad @ g
    k = np.arange(MODES)
    z = np.exp(-2j * np.pi * k / L)                       # (32,)
    zm = z[None, :] ** np.arange(L)[:, None]              # (L, 32)
    Gpre = np.cumsum(zm[:, None, :] * G[:, :, None], axis=0)   # (L, N, 32)
    W = zm[:, None, :] * Gpre[::-1]                       # (L, N, 32) complex
    e = (2.0 - (k == 0)) / L * np.exp(2j * np.pi * k * (PRED_LEN - 1) / L)
    W2 = W * e[None, None, :]

    M = np.concatenate(
        [W2.real.reshape(L, -1), (-W2.imag).reshape(L, -1)], axis=1)
    Uf, sv, Vt = np.linalg.svd(M, full_matrices=False)
    U = Uf[:, :RANK]                                      # (L, r)
    V = sv[:RANK, None] * Vt[:RANK]                       # (r, 32768)

    # fp8 row scaling for V, folded exactly into U; balanced so both
    # U' columns and V' rows sit in fp8 normal range
    alpha = np.sqrt(np.abs(V).max(axis=1) / np.abs(U).max(axis=0))
    u_q = (U * alpha[None, :]).astype(FP8_NP)             # (L, r) fp8
    su = u_q.astype(np.float64).sum(axis=0)               # (r,)
    sun = (-su).astype(BF16_NP)                           # negated row
    Vs = V / alpha[:, None]
    Vre = Vs[:, :N_ORD * MODES]
    Vim = Vs[:, N_ORD * MODES:]

    lch = L // 128
    u_sw = np.ascontiguousarray(
        u_q.reshape(lch, 128, RANK).transpose(1, 0, 2)).reshape(128, -1)
    EmT = Em[-PRED_LEN:].T                                # (N, P)
    emt_sw = np.ascontiguousarray(
        EmT.reshape(2, 128, PRED_LEN).transpose(1, 0, 2)).reshape(
            128, -1).astype(FP8_NP)

    # per-core fp8 vt blocks [KSUB, 128, RANK]
    vt3 = []
    for c in range(N_CORES):
        n0 = c * NSL
        vre = Vre.reshape(RANK, N_ORD, MODES)[:, n0:n0 + NSL, :].reshape(
            RANK, -1)
        vim = Vim.reshape(RANK, N_ORD, MODES)[:, n0:n0 + NSL, :].reshape(
            RANK, -1)
        vt = np.concatenate([vre, vim], axis=1).T         # (2048, r)
        vt3.append(np.ascontiguousarray(
            vt.reshape(KSUB, 128, RANK)).astype(FP8_NP))
    return u_sw, sun, emt_sw, vt3


_CONSTS = None


def _get_consts():
    global _CONSTS
    if _CONSTS is None:
        _CONSTS = [_scale_consts(ms) for ms in MULTISCALE]
    return _CONSTS


# ---------------------------------------------------------------- bass prog
def _build_nc():
    nc = bacc.Bacc("TRN2", target_bir_lowering=False, debug=False,
                   num_devices=N_CORES)

    p = {}
    p["ftx"] = nc.declare_dram_parameter("ftx", [128, SEQ_LEN], FP8,
                                         isOutput=False)
    for s in (0, 1):
        L = (s + 1) * PRED_LEN
        for j in range(NCHUNK):
            p[f"wv{s}c{j}"] = nc.declare_dram_parameter(
                f"wv{s}c{j}", [128, (KSUB // NCHUNK) * (RANK + N_ORD)],
                FP8, isOutput=False)
        p[f"u{s}"] = nc.declare_dram_parameter(
            f"u{s}", [128, (L // 128) * RANK], FP8, isOutput=False)
        p[f"emt{s}"] = nc.declare_dram_parameter(
            f"emt{s}", [128, 2 * PRED_LEN], FP8, isOutput=False)
    p["sun"] = nc.declare_dram_parameter("sun", [1, 2 * BE], BF16,
                                         isOutput=False)
    # host-computed per-series stats: [std, bmu/8] and [ws0, ws1]
    p["sb"] = nc.declare_dram_parameter("sb", [128, 2], F32, isOutput=False)
    p["wsc"] = nc.declare_dram_parameter("wsc", [128, 2], F32,
                                         isOutput=False)
    p["murow"] = nc.declare_dram_parameter("murow", [1, BE], BF16,
                                           isOutput=False)
    p["out_dec"] = nc.declare_dram_parameter("out_dec", [128, PRED_LEN],
                                             F32, isOutput=True)

    with tile.TileContext(nc, num_cores=N_CORES) as tc:
        _emit(nc, tc, p)
    nc.finalize()
    return nc


def _emit(nc, tc, p):
    DR = mybir.MatmulPerfMode.DoubleRow
    MUL = mybir.AluOpType.mult
    ADD = mybir.AluOpType.add
    KC = KSUB // NCHUNK          # 4 ksubs per chunk
    with ExitStack() as ctx:
        const = ctx.enter_context(tc.tile_pool(name="const", bufs=1))
        work = ctx.enter_context(tc.tile_pool(name="work", bufs=1))
        ps_p = ctx.enter_context(
            tc.tile_pool(name="ps_p", bufs=2, space="PSUM"))
        ps_acc = ctx.enter_context(
            tc.tile_pool(name="ps_acc", bufs=2, space="PSUM"))
        ps_dec = ctx.enter_context(
            tc.tile_pool(name="ps_dec", bufs=1, space="PSUM"))

        # tiny host-computed operands on the (software) gpsimd queue
        sb_t = const.tile([128, 2], F32, tag="sb")
        nc.gpsimd.dma_start(sb_t[:], p["sb"][:, :])
        ws_t = const.tile([128, 2], F32, tag="wsc")
        nc.gpsimd.dma_start(ws_t[:], p["wsc"][:, :])
        mu_row = const.tile([1, BE], BF16, tag="murow")
        nc.gpsimd.dma_start(mu_row[:], p["murow"][:, :])
        sun_t = const.tile([1, 2 * BE], BF16, tag="sun")
        nc.gpsimd.dma_start(sun_t[:], p["sun"][:, :])
        # x-path operands (small, fp8) also on gpsimd's software queue so
        # the two HW-DGE queues carry only the big weight stream
        ftx = const.tile([128, SEQ_LEN // 128, BE], FP8, tag="ftx")
        nc.gpsimd.dma_start(ftx[:], p["ftx"][:, :])
        u_t = {}
        u_t[0] = const.tile([128, 4, RANK], FP8, tag="u0", name="u0")
        nc.gpsimd.dma_start(u_t[0][:], p["u0"][:, :])
        u_t[1] = const.tile([128, 8, RANK], FP8, tag="u1", name="u1")
        nc.gpsimd.dma_start(u_t[1][:], p["u1"][:, :])

        # weight streams on the two HW-DGE queues; same-position chunks
        # land together, so each queue carries 2 staggered chunks/scale
        wv = {}
        for j in range(NCHUNK):
            for s in (0, 1):
                wv[s, j] = const.tile([128, KC, RANK + N_ORD], FP8,
                                      tag=f"wv{s}c{j}", name=f"wv{s}c{j}")
        emt_t = {}
        emt_t[0] = const.tile([128, 2, PRED_LEN], FP8, tag="emt0",
                              name="emt0")
        emt_t[1] = const.tile([128, 2, PRED_LEN], FP8, tag="emt1",
                              name="emt1")
        nc.sync.dma_start(wv[0, 0][:], p["wv0c0"][:, :])
        nc.scalar.dma_start(wv[0, 1][:], p["wv0c1"][:, :])
        nc.sync.dma_start(wv[0, 2][:], p["wv0c2"][:, :])
        nc.scalar.dma_start(wv[0, 3][:], p["wv0c3"][:, :])
        nc.sync.dma_start(emt_t[0][:], p["emt0"][:, :])
        nc.scalar.dma_start(emt_t[1][:], p["emt1"][:, :])
        nc.sync.dma_start(wv[1, 0][:], p["wv1c0"][:, :])
        nc.scalar.dma_start(wv[1, 1][:], p["wv1c1"][:, :])
        nc.sync.dma_start(wv[1, 2][:], p["wv1c2"][:, :])
        nc.scalar.dma_start(wv[1, 3][:], p["wv1c3"][:, :])

        # ---- per scale --------------------------------------------------
        dec_ps = ps_dec.tile([BE, PRED_LEN], F32, tag="dec")
        for s in (0, 1):
            lch = (s + 1) * 4
            j0 = SEQ_LEN // 128 - lch

            # P = V@w partial over this core's n-slice (fp8 DoubleRow)
            pps = ps_p.tile([RANK, N_ORD], F32, tag="pps", name=f"pps{s}")
            for j in range(NCHUNK):
                for kk in (0, 2):
                    nc.tensor.matmul(
                        pps[:],
                        lhsT=wv[s, j][:, kk:kk + 2, 0:RANK],
                        rhs=wv[s, j][:, kk:kk + 2, RANK:RANK + N_ORD],
                        start=(j == 0 and kk == 0),
                        stop=(j == NCHUNK - 1 and kk == 2),
                        perf_mode=DR)
            p_sb = work.tile([RANK, N_ORD], BF16, tag=f"p{s}", name=f"p{s}")
            nc.vector.tensor_copy(p_sb[:], pps[:])

            # gT = U.T @ f - su x mu  (transposed; norm correction folded)
            gT_ps = ps_acc.tile([RANK, BE], F32, tag="acc", name=f"gT{s}")
            for dd in range(0, lch, 2):
                nc.tensor.matmul(gT_ps[:], lhsT=u_t[s][:, dd:dd + 2, :],
                                 rhs=ftx[:, j0 + dd:j0 + dd + 2, :],
                                 start=(dd == 0), stop=False, perf_mode=DR)
            nc.tensor.matmul(gT_ps[:], lhsT=sun_t[:, s * BE:(s + 1) * BE],
                             rhs=mu_row[:], start=False, stop=True)
            gT_sb = work.tile([RANK, BE], BF16, tag=f"gT{s}",
                              name=f"gTs{s}")
            nc.vector.tensor_copy(gT_sb[:], gT_ps[:])

            # xdcT[o, be] = P.T @ gTc   (built transposed)
            xdcT_ps = ps_acc.tile([128, 2, BE], F32, tag="acc",
                                  name=f"xdcT{s}")
            for och in (0, 1):
                nc.tensor.matmul(
                    xdcT_ps[:, och, :],
                    lhsT=p_sb[:, och * 128:(och + 1) * 128],
                    rhs=gT_sb[:], start=True, stop=True)
            xdcT_sb = work.tile([128, 2, BE], BF16, tag=f"xdcT{s}",
                                name=f"xdcTs{s}")
            nc.vector.tensor_scalar_mul(xdcT_sb[:], xdcT_ps[:],
                                        ws_t[:, s:s + 1])

            # dec[be, p] += ws * xdcT.T @ EmT
            for och in (0, 1):
                nc.tensor.matmul(
                    dec_ps[:],
                    lhsT=xdcT_sb[:, och, :],
                    rhs=emt_t[s][:, och, :],
                    start=(s == 0 and och == 0),
                    stop=(s == 1 and och == 1))

        # ---- final per-series affine on DVE, split to overlap store -----
        out_sb = work.tile([BE, PRED_LEN], F32, tag="out")
        for oh, eng in ((0, nc.sync), (1, nc.scalar)):
            sl = slice(oh * (PRED_LEN // 2), (oh + 1) * (PRED_LEN // 2))
            nc.vector.tensor_scalar(out_sb[:, sl], dec_ps[:, sl],
                                    sb_t[:, 0:1], sb_t[:, 1:2],
                                    op0=MUL, op1=ADD)
            eng.dma_start(p["out_dec"][:, sl], out_sb[:, sl])


_NC = None


def _get_nc():
    global _NC
    if _NC is None:
        _NC = _build_nc()
    return _NC


# ---------------------------------------------------------------- host side
def _in_maps(x_enc, spec_w_real, spec_w_imag, mlp_weight, mlp_bias):
    consts = _get_consts()

    xt = np.transpose(x_enc, (0, 2, 1)).reshape(BE, SEQ_LEN).astype(
        np.float64)
    mean = xt.mean(axis=1)
    std = np.sqrt(xt.var(axis=1) + 1e-5)
    ftx = np.ascontiguousarray(
        x_enc.transpose(1, 0, 2).reshape(SEQ_LEN, BE)
        .reshape(8, 128, BE).transpose(1, 0, 2)).reshape(128, -1).astype(
            FP8_NP)

    S_w = []
    wt3 = {}
    for s in (0, 1):
        wmax = max(np.abs(spec_w_real[s]).max(),
                   np.abs(spec_w_imag[s]).max(), 1e-30)
        sw = 224.0 / wmax
        S_w.append(sw)
        for c in range(N_CORES):
            n0 = c * NSL
            wre = (spec_w_real[s, n0:n0 + NSL] * sw).transpose(
                0, 2, 1).reshape(-1, N_ORD)
            wim = (spec_w_imag[s, n0:n0 + NSL] * sw).transpose(
                0, 2, 1).reshape(-1, N_ORD)
            wt = np.concatenate([wre, wim], axis=0)       # (2048, 256)
            wt3[s, c] = wt.reshape(KSUB, 128, N_ORD).astype(FP8_NP)

    bmu8 = (float(mlp_bias[0]) * std + mean) / N_CORES
    sb = np.stack([std, bmu8], axis=1).astype(np.float32)     # (128, 2)
    wsc = np.broadcast_to(
        np.array([float(mlp_weight[0, 0]) / S_w[0],
                  float(mlp_weight[0, 1]) / S_w[1]], np.float32),
        (128, 2)).copy()
    murow = np.ascontiguousarray(mean.reshape(1, BE)).astype(BF16_NP)
    sun = np.concatenate([consts[0][1], consts[1][1]]).reshape(1, -1)

    shared = {"ftx": ftx, "sb": sb, "wsc": wsc, "murow": murow,
              "sun": np.ascontiguousarray(sun)}
    for s in (0, 1):
        shared[f"u{s}"] = consts[s][0]
        shared[f"emt{s}"] = consts[s][2]

    KC = KSUB // NCHUNK
    maps = []
    for c in range(N_CORES):
        m = dict(shared)
        for s in (0, 1):
            arr = np.concatenate([consts[s][3][c], wt3[s, c]], axis=2)
            arr = np.ascontiguousarray(
                arr.transpose(1, 0, 2)).reshape(128, -1)
            w = KC * (RANK + N_ORD)
            for j in range(NCHUNK):
                m[f"wv{s}c{j}"] = np.ascontiguousarray(
                    arr[:, j * w:(j + 1) * w])
        maps.append(m)
    return maps


def kernel(x_enc, spec_w_real, spec_w_imag, mlp_weight, mlp_bias,
           _trace=False, _trace_kwargs=None):
    x_enc = np.asarray(x_enc, np.float32)
    spec_w_real = np.asarray(spec_w_real, np.float32)
    spec_w_imag = np.asarray(spec_w_imag, np.float32)
    mlp_weight = np.asarray(mlp_weight, np.float32).reshape(1, 2)
    mlp_bias = np.asarray(mlp_bias, np.float32).reshape(1)
    maps = _in_maps(x_enc, spec_w_real, spec_w_imag, mlp_weight, mlp_bias)
    nc = _get_nc()
    res = run_bass_kernel_spmd(nc, maps, list(range(N_CORES)),
                               trace=_trace, **(_trace_kwargs or {}))
    # out_dec[c] = partial dec over core c's n-shard; unshard = sum
    full = np.sum([res.results[c]["out_dec"] for c in range(N_CORES)],
                  axis=0, dtype=np.float32)
    out = np.ascontiguousarray(
        full.reshape(B_SZ, E_IN, PRED_LEN).transpose(0, 2, 1), np.float32)
    if _trace:
        return out, res
    return out



# revision 3
# speedup vs baseline: 1.2678x; 1.2678x over previous
"""Trainium2 Bass kernel for nn_Model_17085379903564 (HiPPO-LegT multiscale
spectral forecaster).

Math: the reference normalizes x per (b,e) series, runs a HiPPO-LegT scan,
takes 32 rFFT modes of the state trajectory, mixes modes with complex
weights w, evaluates the irFFT at t=511, projects on Legendre polynomials
(Em), mixes two scales with an MLP, and un-normalizes.

Everything from the input to the Legendre projection is LINEAR with
constant coefficients, so per scale (L = 512 or 1024) the whole chain
collapses to one dense operator W2 folded from the scan kernel, the DFT
and the point-irFFT weights.  W2 is factored by SVD, W2 ~= U @ V; with
g = f.T @ U and P = V @ w (x-independent), xdc = g @ P.  The real
input/weight distributions excite only the top singular directions;
RANK=32 measures within ~1e-4 of RANK=128 end to end (quantization noise
dominates), far inside the 2e-2 gate.

Quantization: V rows are scaled to fp8 range with per-row factors folded
into U (exact algebra); w is scaled globally per scale with the factor
folded into the host-applied mlp weight.  P is computed with fp8
DoubleRow matmuls.  The instance norm is handled by subtracting the
host-computed per-series mean from x BEFORE fp8 quantization (exact in
f64), and the /std plus mean restoration is a host-side per-series
affine applied after summing the per-core partials — so the device
pipeline is purely linear: gT = U'.T @ f; P = V' @ w'; xdcT = P.T @ gT;
dec = xdcT.T @ EmT; store dec as bf16.

Sharding (8 cores): scale-parallel x spectral-parallel.  Cores 0-3 own
scale 0 (L=512), cores 4-7 own scale 1 (L=1024); within a scale group
each core takes 64 of the 256 spectral input rows n (the contraction
axis of V@w), producing a full-size partial dec that the host sums.
The program is identical on all cores (SPMD); scale 0's shorter window
is realized by zero rows in U' (exact).

DMA layout: per-core traffic is ~1.6 MB, all in partition-major [128, N]
lines of >=512B so no descriptor-size penalty applies.  The weight
stream is split into few large chunks across the two HWDGE queues
(sync/scalar) to minimize per-DMA issue overhead; the x-path (xu) loads
first so the gT matmuls fill the PE while weights stream; the last
weight chunk on the critical path is tiny (2 k-subtiles) so the
post-stream dependency tail is short.  PSUM->SBUF copies are split
between the Vector and Activation engines; the dec output is stored as
two bf16 halves so the first store overlaps the second half's compute.
"""

from contextlib import ExitStack

import ml_dtypes
import numpy as np

import concourse.bacc as bacc
import concourse.bass as bass
import concourse.mybir as mybir
import concourse.tile as tile
from concourse.bass_utils import run_bass_kernel_spmd

# ---- problem constants (hardcoded; kernel.py must be self-contained) ----
B_SZ = 4
SEQ_LEN = 1024
PRED_LEN = 512
E_IN = 32
N_ORD = 256
MODES = 32
MULTISCALE = (1, 2)
BE = B_SZ * E_IN            # 128
N_CORES = 8
GRP = 4                     # cores per scale
NSL = N_ORD // GRP          # 64  n-rows per core
NK = 2 * NSL * MODES        # 4096 contraction length per core (re+im)
KSUB = NK // 128            # 32 k-subtiles
RANK = 32                   # SVD rank kept for the W2 operators
# weight chunks: (name, ksub range).  a/b/c stream on sync in consumption
# order with a tiny tail chunk c; d streams on scalar and lands early.
KCH = (("wva", 0, 12), ("wvb", 12, 22), ("wvc", 22, 24), ("wvd", 24, 32))

F32 = mybir.dt.float32
BF16 = mybir.dt.bfloat16
FP8 = mybir.dt.float8e4
BF16_NP = np.dtype(ml_dtypes.bfloat16)
FP8_NP = np.dtype(ml_dtypes.float8_e4m3)


# ---------------------------------------------------------------- constants
def _transition_lmu(N):
    Q = np.arange(N, dtype=np.float64)
    R = (2 * Q + 1)[:, None]
    j, i = np.meshgrid(Q, Q)
    A = np.where(i < j, -1.0, (-1.0) ** (i - j + 1)) * R
    Bv = ((-1.0) ** Q[:, None] * R)[:, 0]
    return A, Bv


def _bilinear(A, Bv, dt):
    I = np.eye(A.shape[0])
    M = I - (dt / 2.0) * A
    Ad = np.linalg.solve(M, I + (dt / 2.0) * A)
    Bd = np.linalg.solve(M, dt * Bv)
    return Ad, Bd


def _legendre_vander(x, N):
    P = np.zeros((N, x.shape[0]))
    P[0] = 1.0
    if N > 1:
        P[1] = x
    for n in range(1, N - 1):
        P[n + 1] = ((2 * n + 1) * x * P[n] - n * P[n - 1]) / (n + 1)
    return P.T


def _scale_consts(ms):
    """Per-scale constants: u (zero-padded to the 1024 frame, swizzled),
    the per-shard fp8 V blocks and the swizzled Em eval matrix."""
    L = ms * PRED_LEN
    A, Bv = _transition_lmu(N_ORD)
    Ad, Bd = _bilinear(A, Bv, 1.0 / L)
    vals = np.arange(0.0, 1.0, 1.0 / L)
    Em = _legendre_vander(1.0 - 2.0 * vals, N_ORD)         # (L, N)

    G = np.empty((L, N_ORD))
    g = Bd.copy()
    for m in range(L):
        G[m] = g
        g = Ad @ g
    k = np.arange(MODES)
    z = np.exp(-2j * np.pi * k / L)                        # (32,)
    zm = z[None, :] ** np.arange(L)[:, None]               # (L, 32)
    Gpre = np.cumsum(zm[:, None, :] * G[:, :, None], axis=0)    # (L, N, 32)
    W = zm[:, None, :] * Gpre[::-1]                        # (L, N, 32) complex
    e = (2.0 - (k == 0)) / L * np.exp(2j * np.pi * k * (PRED_LEN - 1) / L)
    W2 = W * e[None, None, :]

    M = np.concatenate(
        [W2.real.reshape(L, -1), (-W2.imag).reshape(L, -1)], axis=1)
    Uf, sv, Vt = np.linalg.svd(M, full_matrices=False)
    U = Uf[:, :RANK]                                       # (L, r)
    V = sv[:RANK, None] * Vt[:RANK]                        # (r, 16384)

    # fp8 row scaling for V, folded exactly into U; balanced so both
    # U' columns and V' rows sit in fp8 normal range
    alpha = np.sqrt(np.abs(V).max(axis=1) / np.abs(U).max(axis=0))
    u_q = (U * alpha[None, :]).astype(FP8_NP)              # (L, r) fp8
    u_f = np.zeros((SEQ_LEN, RANK), FP8_NP)
    u_f[SEQ_LEN - L:] = u_q                                # embed in 1024 frame
    u_sw = np.ascontiguousarray(
        u_f.reshape(8, 128, RANK).transpose(1, 0, 2))      # [128, 8, r]

    Vs = V / alpha[:, None]
    Vre = Vs[:, :N_ORD * MODES]
    Vim = Vs[:, N_ORD * MODES:]
    vt3 = []
    for cc in range(GRP):
        n0 = cc * NSL
        vre = Vre.reshape(RANK, N_ORD, MODES)[:, n0:n0 + NSL, :].reshape(
            RANK, -1)
        vim = Vim.reshape(RANK, N_ORD, MODES)[:, n0:n0 + NSL, :].reshape(
            RANK, -1)
        vt = np.concatenate([vre, vim], axis=1).T          # (4096, r)
        vt3.append(np.ascontiguousarray(
            vt.reshape(KSUB, 128, RANK)).astype(FP8_NP))

    EmT = Em[-PRED_LEN:].T                                 # (N, P)
    emt_sw = np.ascontiguousarray(
        EmT.reshape(2, 128, PRED_LEN).transpose(1, 0, 2)).astype(FP8_NP)
    return u_sw, vt3, emt_sw


_CONSTS = None


def _get_consts():
    global _CONSTS
    if _CONSTS is None:
        _CONSTS = [_scale_consts(ms) for ms in MULTISCALE]
    return _CONSTS


# ---------------------------------------------------------------- bass prog
def _build_nc():
    nc = bacc.Bacc("TRN2", target_bir_lowering=False, debug=False,
                   num_devices=N_CORES)

    p = {}
    p["xu"] = nc.declare_dram_parameter("xu", [128, 8 * (BE + RANK)], FP8,
                                        isOutput=False)
    for name, k0, k1 in KCH:
        p[name] = nc.declare_dram_parameter(
            name, [128, (k1 - k0) * (RANK + N_ORD)], FP8, isOutput=False)
    p["emt"] = nc.declare_dram_parameter("emt", [128, 2 * PRED_LEN], FP8,
                                         isOutput=False)
    p["out"] = nc.declare_dram_parameter("out", [128, PRED_LEN], BF16,
                                         isOutput=True)

    with tile.TileContext(nc, num_cores=N_CORES) as tc:
        _emit(nc, tc, p)
    nc.finalize()
    return nc


def _emit(nc, tc, p):
    DR = mybir.MatmulPerfMode.DoubleRow
    with ExitStack() as ctx:
        const = ctx.enter_context(tc.tile_pool(name="const", bufs=1))
        work = ctx.enter_context(tc.tile_pool(name="work", bufs=1))
        ps_g = ctx.enter_context(
            tc.tile_pool(name="ps_g", bufs=1, space="PSUM"))
        ps_p = ctx.enter_context(
            tc.tile_pool(name="ps_p", bufs=1, space="PSUM"))
        ps_x = ctx.enter_context(
            tc.tile_pool(name="ps_x", bufs=1, space="PSUM"))
        ps_d = ctx.enter_context(
            tc.tile_pool(name="ps_d", bufs=1, space="PSUM"))

        # x-path first on sync so gT fills the PE while weights stream;
        # weight chunks a/b on sync in consumption order with the tiny
        # tail chunk c; d + emt on the scalar HWDGE queue (land early).
        xu = const.tile([128, 8, BE + RANK], FP8, tag="xu")
        nc.sync.dma_start(xu[:], p["xu"][:, :])
        wv = {}
        for name, k0, k1 in KCH:
            wv[name] = const.tile([128, k1 - k0, RANK + N_ORD], FP8,
                                  tag=name, name=name)
        nc.scalar.dma_start(wv["wvd"][:], p["wvd"][:, :])
        nc.sync.dma_start(wv["wva"][:], p["wva"][:, :])
        nc.scalar.dma_start(wv["wvb"][:], p["wvb"][:, :])
        nc.sync.dma_start(wv["wvc"][:], p["wvc"][:, :])
        emt = const.tile([128, 2, PRED_LEN], FP8, tag="emt")
        nc.scalar.dma_start(emt[:], p["emt"][:, :])

        # gT[r, be] = sum_t U'[t, r] (x - mu)[be, t]  (fp8 DoubleRow)
        gT_ps = ps_g.tile([RANK, BE], F32, tag="gt")
        for dd in range(0, 8, 2):
            nc.tensor.matmul(gT_ps[:], lhsT=xu[:, dd:dd + 2, BE:BE + RANK],
                             rhs=xu[:, dd:dd + 2, 0:BE],
                             start=(dd == 0), stop=(dd == 6), perf_mode=DR)
        gT_sb = work.tile([RANK, BE], BF16, tag="gt")
        # first Activation-engine op: absorbs the act-table load early
        nc.scalar.copy(gT_sb[:], gT_ps[:])

        # P[r, n] = sum_k V'[k, r] w'[k, n], accumulated over k-subtile
        # chunks in expected arrival order (d early on scalar, then a, b,
        # and the tiny tail c)
        pps = ps_p.tile([RANK, N_ORD], F32, tag="pps")
        order = ("wvd", "wva", "wvb", "wvc")
        for ci, name in enumerate(order):
            ks = dict((n, (a, b)) for n, a, b in KCH)[name]
            nks = ks[1] - ks[0]
            for kk in range(0, nks, 2):
                nc.tensor.matmul(
                    pps[:],
                    lhsT=wv[name][:, kk:kk + 2, 0:RANK],
                    rhs=wv[name][:, kk:kk + 2, RANK:RANK + N_ORD],
                    start=(ci == 0 and kk == 0),
                    stop=(ci == 3 and kk == nks - 2),
                    perf_mode=DR)
        p_sb = work.tile([RANK, N_ORD], BF16, tag="p")
        nc.vector.tensor_copy(p_sb[:, 0:128], pps[:, 0:128])
        nc.scalar.copy(p_sb[:, 128:256], pps[:, 128:256])

        # xdcT[n, be] = sum_r P[r, n] gT[r, be]
        xd_ps = ps_x.tile([128, 2, BE], F32, tag="xd")
        for h in (0, 1):
            nc.tensor.matmul(xd_ps[:, h, :],
                             lhsT=p_sb[:, h * 128:(h + 1) * 128],
                             rhs=gT_sb[:], start=True, stop=True)
        xd_sb = work.tile([128, 2, BE], BF16, tag="xd")
        nc.vector.tensor_copy(xd_sb[:, 0, :], xd_ps[:, 0, :])
        nc.scalar.copy(xd_sb[:, 1, :], xd_ps[:, 1, :])

        # dec[be, l] = sum_n xdcT[n, be] EmT[n, l], column-split so the
        # first half stores while the second half computes
        dec_ps = ps_d.tile([BE, 2, 256], F32, tag="dec")
        out_sb = work.tile([BE, 2, 256], BF16, tag="out")
        for lh in (0, 1):
            for h in (0, 1):
                nc.tensor.matmul(
                    dec_ps[:, lh, :],
                    lhsT=xd_sb[:, h, :],
                    rhs=emt[:, h, lh * 256:(lh + 1) * 256],
                    start=(h == 0), stop=(h == 1))
            if lh == 0:
                nc.vector.tensor_copy(out_sb[:, 0, :], dec_ps[:, 0, :])
                nc.sync.dma_start(p["out"][:, 0:256], out_sb[:, 0, :])
            else:
                nc.scalar.copy(out_sb[:, 1, :], dec_ps[:, 1, :])
                nc.scalar.dma_start(p["out"][:, 256:512], out_sb[:, 1, :])


_NC = None


def _get_nc():
    global _NC
    if _NC is None:
        _NC = _build_nc()
    return _NC


# ---------------------------------------------------------------- host side
def _in_maps(x_enc, spec_w_real, spec_w_imag):
    consts = _get_consts()

    xt = np.transpose(x_enc, (0, 2, 1)).reshape(BE, SEQ_LEN).astype(
        np.float64)
    mean = xt.mean(axis=1)
    std = np.sqrt(xt.var(axis=1) + 1e-5)
    ftc = (xt - mean[:, None]).astype(FP8_NP)              # (BE, 1024)
    ftx_sw = np.ascontiguousarray(
        ftc.T.reshape(8, 128, BE).transpose(1, 0, 2))      # [128, 8, BE]

    S_w = []
    maps = []
    for s in (0, 1):
        wmax = max(np.abs(spec_w_real[s]).max(),
                   np.abs(spec_w_imag[s]).max(), 1e-30)
        S_w.append(224.0 / wmax)
    for c in range(N_CORES):
        s, cc = c // GRP, c % GRP
        u_sw, vt3, emt_sw = consts[s]
        sw = S_w[s]
        n0 = cc * NSL
        wre = (spec_w_real[s, n0:n0 + NSL] * sw).transpose(
            0, 2, 1).reshape(-1, N_ORD)
        wim = (spec_w_imag[s, n0:n0 + NSL] * sw).transpose(
            0, 2, 1).reshape(-1, N_ORD)
        wt = np.concatenate([wre, wim], axis=0).reshape(
            KSUB, 128, N_ORD).astype(FP8_NP)
        arr = np.concatenate([vt3[cc], wt], axis=2)        # [KSUB,128,288]
        arr = np.ascontiguousarray(arr.transpose(1, 0, 2)) # [128,KSUB,288]
        m = {
            "xu": np.ascontiguousarray(
                np.concatenate([ftx_sw, u_sw], axis=2)).reshape(128, -1),
            "emt": emt_sw.reshape(128, -1),
        }
        for name, k0, k1 in KCH:
            m[name] = np.ascontiguousarray(
                arr[:, k0:k1, :]).reshape(128, -1)
        maps.append(m)
    return maps, S_w, mean, std


def kernel(x_enc, spec_w_real, spec_w_imag, mlp_weight, mlp_bias,
           _trace=False, _trace_kwargs=None):
    x_enc = np.asarray(x_enc, np.float32)
    spec_w_real = np.asarray(spec_w_real, np.float32)
    spec_w_imag = np.asarray(spec_w_imag, np.float32)
    mlp_weight = np.asarray(mlp_weight, np.float32).reshape(1, 2)
    mlp_bias = np.asarray(mlp_bias, np.float32).reshape(1)
    maps, S_w, mean, std = _in_maps(x_enc, spec_w_real, spec_w_imag)
    nc = _get_nc()
    res = run_bass_kernel_spmd(nc, maps, list(range(N_CORES)),
                               trace=_trace, **(_trace_kwargs or {}))
    # out[c] = partial dec over core c's scale/n-shard; unshard = scaled sum
    # plus the host-side per-series affine (std scaling + mean restore)
    dec = np.zeros((BE, PRED_LEN), np.float64)
    for c in range(N_CORES):
        s = c // GRP
        part = np.asarray(res.results[c]["out"]).astype(np.float64)
        dec += (float(mlp_weight[0, s]) / S_w[s]) * part
    out = dec * std[:, None] + (float(mlp_bias[0]) * std + mean)[:, None]
    out = np.ascontiguousarray(
        out.reshape(B_SZ, E_IN, PRED_LEN).transpose(0, 2, 1)).astype(
            np.float32)
    if _trace:
        return out, res
    return out
